# revision 1
# baseline (speedup 1.0000x reference)
"""Decoder block (single-head causal attention + GELU FFN) on 8 TRN2 NeuronCores.

Sharding: pure data parallel, no collectives. Core c handles batch b = c//2 and
1024 query tokens of that batch, chosen as two 512-token chunks that balance the
causal-attention workload:
  even cores (half 0): chunks 0 and 3  (rows    0:512  and 1536:2048)
  odd  cores (half 1): chunks 1 and 2  (rows  512:1024 and 1024:1536)
The SPMD program is identical on every core (run_bass_kernel_spmd compiles one
program); all per-core differences are data (which tokens are in x_own, qpos
values that drive on-chip causal-mask generation).

Layout convention: feature-major ("transposed") everywhere on chip. The host
pre-transposes x and all weights, and re-transposes the output, so the kernel
needs no on-chip transposes.

Precision: matmuls run as float32r (full-rate fp32 PE mode) except the
attention probs @ V product, where probs/V are stored fp16 (fp32 PSUM accum).
"""

import numpy as np

D = 1024  # model dim
S = 2048  # sequence length
B = 4  # batch
M = 4096  # FFN dim
CH = 512  # q chunk (slot) size
NDT = D // 128  # 8 d-tiles
N_CORES = 8

_PROGRAM = None  # cached compiled program


def _build_program():
    import sys

    if "/opt/trn_rl_repo" not in sys.path:
        sys.path.insert(0, "/opt/trn_rl_repo")
    import concourse.bass as bass
    import concourse.tile as tile
    import concourse.mybir as mybir
    from concourse import bacc
    from concourse.bass import ts

    dt = mybir.dt
    AF = mybir.ActivationFunctionType
    ALU = mybir.AluOpType
    F32, BF16, F32R, F16 = dt.float32, dt.bfloat16, dt.float32r, dt.float16

    nc = bacc.Bacc("TRN2", target_bir_lowering=False, debug=False)

    # ---------------- DRAM I/O ----------------
    xT = nc.dram_tensor("xT", [D, S], F32R, kind="ExternalInput").ap()
    xoT = nc.dram_tensor("xoT", [D, 2 * CH], F32R, kind="ExternalInput").ap()
    wqT = nc.dram_tensor("wqT", [D, D], F32R, kind="ExternalInput").ap()
    wkT = nc.dram_tensor("wkT", [D, D], F32R, kind="ExternalInput").ap()
    wvT = nc.dram_tensor("wvT", [D, D], F32R, kind="ExternalInput").ap()
    woT = nc.dram_tensor("woT", [D, D], F32R, kind="ExternalInput").ap()
    wfT = nc.dram_tensor("wfT", [D, M], F32R, kind="ExternalInput").ap()
    bq = nc.dram_tensor("bq", [128, D // 128], F32, kind="ExternalInput").ap()
    bk = nc.dram_tensor("bk", [128, D // 128], F32, kind="ExternalInput").ap()
    bo2 = nc.dram_tensor("bo2", [128, D // 128], F32, kind="ExternalInput").ap()
    bfT = nc.dram_tensor("bfT", [128, M // 128], F32, kind="ExternalInput").ap()
    qpos = nc.dram_tensor("qpos", [1, 2 * CH], F32R, kind="ExternalInput").ap()
    iota_kt = nc.dram_tensor("iota_kt", [128, S // 128], F32, kind="ExternalInput").ap()
    ffT = nc.dram_tensor("ffT", [M, 2 * CH], F32, kind="ExternalOutput").ap()

    vS = nc.dram_tensor("vS", [S, D], F16).ap()  # V scratch, token-major, fp16

    NKT = [8, 16]  # k-tiles per slot (slotA: k<1024, slotB: k<2048)

    with tile.TileContext(nc) as tc:
        with (
            tc.tile_pool(name="const", bufs=1) as cpool,
            tc.tile_pool(name="persist", bufs=1) as ppool,
            tc.tile_pool(name="psum", bufs=1, space="PSUM") as pspool,
        ):
            # ---------------- constants ----------------
            ones_col_bf = cpool.tile([128, 1], F16, name="ones_col_bf", tag="ones_col_bf")
            nc.vector.memset(ones_col_bf[:], 1.0)
            ones_row_f = cpool.tile([1, 128], F32, name="ones_row_f", tag="ones_row_f")
            nc.vector.memset(ones_row_f[:], 1.0)
            ones_row = cpool.tile([1, 128], F32R, name="ones_row", tag="ones_row")
            nc.vector.tensor_copy(ones_row[:], ones_row_f[:])
            iota_sb = cpool.tile([128, S // 128], F32, name="iota", tag="iota")
            nc.sync.dma_start(iota_sb[:], iota_kt[:])
            bq_sb = cpool.tile([128, D // 128], F32, name="bq", tag="bq")
            nc.sync.dma_start(bq_sb[:], bq[:])
            bk_sb = cpool.tile([128, D // 128], F32, name="bk", tag="bk")
            nc.sync.dma_start(bk_sb[:], bk[:])
            bo2_sb = cpool.tile([128, D // 128], F32, name="bo2", tag="bo2")
            nc.sync.dma_start(bo2_sb[:], bo2[:])
            bf_sb = cpool.tile([128, M // 128], F32, name="bf", tag="bf")
            nc.sync.dma_start(bf_sb[:], bfT[:])
            qpos_row = cpool.tile([1, 2 * CH], F32R, name="qpos_row", tag="qpos_row")
            nc.sync.dma_start(qpos_row[:], qpos[:])

            # broadcast qpos to 128 partitions via ones outer-product
            qposB = cpool.tile([128, 2 * CH], F32, name="qposB", tag="qposB")
            for i in range(2 * CH // 512):
                bc_ps = pspool.tile([128, 512], F32, name="small", tag="small", bufs=1)
                nc.tensor.matmul(
                    bc_ps[:], ones_row[:], qpos_row[:, ts(i, 512)],
                    start=True, stop=True,
                )
                nc.scalar.activation(qposB[:, ts(i, 512)], bc_ps[:], AF.Copy)

            # ---------------- P1 + P2 ----------------
            attnT = [
                [ppool.tile([128, CH], F32R, name=f"at{dt_}_{c}", tag=f"at{dt_}_{c}") for c in range(2)]
                for dt_ in range(NDT)
            ]
            with tc.tile_pool(name="ktpool", bufs=1) as ktp:
                # K^T stays resident in SBUF for the whole attention phase
                kT = [ktp.tile([128, S], F32R, name=f"kT{i}", tag=f"kT{i}") for i in range(NDT)]

                with tc.tile_pool(name="p1a", bufs=1) as p1a:
                    wk_sb = [p1a.tile([128, D], F32R, name=f"wk{i}", tag=f"wk{i}") for i in range(NDT)]
                    wv_sb = [p1a.tile([128, D], F32R, name=f"wv{i}", tag=f"wv{i}") for i in range(NDT)]
                    for h in range(2):
                        for i in range(NDT):
                            nc.sync.dma_start(
                                wk_sb[i][:, ts(h, 512)], wkT[ts(i, 128), ts(h, 512)]
                            )
                    for tb in range(S // 512):  # K pass: psum -> resident kT
                        xblk = [p1a.tile([128, 512], F32R, name=f"xa{i}", tag=f"xa{i}", bufs=2) for i in range(NDT)]
                        for i in range(NDT):
                            nc.sync.dma_start(xblk[i][:], xT[ts(i, 128), ts(tb, 512)])
                        for ot in range(NDT):
                            ps = pspool.tile([128, 512], F32, name="mm", tag="mm", bufs=3)
                            for i in range(NDT):
                                nc.tensor.matmul(
                                    ps[:], wk_sb[i][:, ts(ot, 128)], xblk[i][:],
                                    start=(i == 0), stop=(i == NDT - 1),
                                )
                            nc.scalar.activation(
                                kT[ot][:, ts(tb, 512)], ps[:], AF.Identity,
                                bias=bk_sb[:, ot : ot + 1],
                            )
                        if tb < 2:  # stagger wv loads behind the x stream
                            for i in range(NDT):
                                nc.sync.dma_start(
                                    wv_sb[i][:, ts(tb, 512)], wvT[ts(i, 128), ts(tb, 512)]
                                )
                    for tb in range(S // 512):  # V pass (re-reads x)
                        xblk = [p1a.tile([128, 512], F32R, name=f"xa{i}", tag=f"xa{i}", bufs=2) for i in range(NDT)]
                        for i in range(NDT):
                            nc.sync.dma_start(xblk[i][:], xT[ts(i, 128), ts(tb, 512)])
                        for tt in range(4):
                            stv = p1a.tile([128, D], F16, name="vstage", tag="vstage", bufs=3)
                            for ob in range(2):
                                ps = pspool.tile([128, 512], F32, name="mm", tag="mm", bufs=3)
                                for i in range(NDT):
                                    nc.tensor.matmul(
                                        ps[:],
                                        xblk[i][:, ts(tt, 128)],
                                        wv_sb[i][:, ts(ob, 512)],
                                        start=(i == 0), stop=(i == NDT - 1),
                                    )
                                nc.scalar.activation(stv[:, ts(ob, 512)], ps[:], AF.Copy)
                            nc.sync.dma_start(vS[ts(tb * 4 + tt, 128), :], stv[:])

                with tc.tile_pool(name="qtpool", bufs=1) as qtp:
                    qT = [
                        [qtp.tile([128, CH], F32R, name=f"qT{dt_}_{qb}", tag=f"qT{dt_}_{qb}") for qb in range(2)]
                        for dt_ in range(NDT)
                    ]
                    with tc.tile_pool(name="p1c", bufs=1) as p1c:
                        wq_sb = [p1c.tile([128, D], F32R, name=f"wq{i}", tag=f"wq{i}") for i in range(NDT)]
                        for h in range(2):
                            for i in range(NDT):
                                nc.sync.dma_start(
                                    wq_sb[i][:, ts(h, 512)], wqT[ts(i, 128), ts(h, 512)]
                                )
                        for qb in range(2):
                            xblk = [p1c.tile([128, 512], F32R, name=f"xc{i}", tag=f"xc{i}", bufs=2) for i in range(NDT)]
                            for i in range(NDT):
                                nc.sync.dma_start(xblk[i][:], xoT[ts(i, 128), ts(qb, 512)])
                            for ot in range(NDT):
                                ps = pspool.tile([128, 512], F32, name="mm", tag="mm", bufs=3)
                                for i in range(NDT):
                                    nc.tensor.matmul(
                                        ps[:], wq_sb[i][:, ts(ot, 128)], xblk[i][:],
                                        start=(i == 0), stop=(i == NDT - 1),
                                    )
                                nc.scalar.activation(
                                    qT[ot][qb][:], ps[:], AF.Identity, bias=bq_sb[:, ot : ot + 1]
                                )

                    # ---------------- P2: attention ----------------
                    with tc.tile_pool(name="p2", bufs=1) as p2:
                        vt = [
                            p2.tile([128, D], F16, name=f"v{k}", tag=f"v{k}", bufs=1)
                            for k in range(16)
                        ]
                        for k in range(16):
                            nc.sync.dma_start(vt[k][:], vS[ts(k, 128), :])
                        for ch in range(2):
                            nkt = NKT[ch]
                            pt = [
                                p2.tile([128, CH], F16, name=f"pt{k}", tag=f"pt{k}", bufs=1)
                                for k in range(nkt)
                            ]
                            dn_ps = pspool.tile([1, CH], F32, name="small", tag="small", bufs=1)
                            for k in range(nkt):
                                ps = pspool.tile([128, CH], F32, name="mm", tag="mm", bufs=3)
                                for i in range(NDT):
                                    nc.tensor.matmul(
                                        ps[:],
                                        kT[i][:, ts(k, 128)],
                                        qT[i][ch][:],
                                        start=(i == 0), stop=(i == NDT - 1),
                                    )
                                masked = (ch == 0) or (k >= 8)
                                if masked:
                                    praw = p2.tile([128, CH], F16, name="praw", tag="praw", bufs=2)
                                    nc.scalar.activation(
                                        praw[:], ps[:], AF.Exp, scale=1.0 / 32.0
                                    )
                                    msk = p2.tile([128, CH], F16, name="msk", tag="msk", bufs=2)
                                    nc.vector.tensor_scalar(
                                        out=msk[:],
                                        in0=qposB[:, ts(ch, CH)],
                                        scalar1=iota_sb[:, k : k + 1],
                                        scalar2=None,
                                        op0=ALU.is_ge,
                                    )
                                    nc.vector.tensor_tensor(
                                        out=pt[k][:], in0=praw[:], in1=msk[:],
                                        op=ALU.mult,
                                    )
                                else:
                                    nc.scalar.activation(
                                        pt[k][:], ps[:], AF.Exp, scale=1.0 / 32.0
                                    )
                                nc.tensor.matmul(
                                    dn_ps[:], ones_col_bf[:], pt[k][:],
                                    start=(k == 0), stop=(k == nkt - 1),
                                )
                            # 1/denom, broadcast to 128 partitions
                            recip = p2.tile([1, CH], F32, name="recip", tag="recip", bufs=2)
                            nc.vector.reciprocal(recip[:], dn_ps[:])
                            recip_r = p2.tile([1, CH], F32R, name="recip_r", tag="recip_r", bufs=2)
                            nc.vector.tensor_copy(recip_r[:], recip[:])
                            rb_ps = pspool.tile([128, CH], F32, name="small", tag="small", bufs=1)
                            nc.tensor.matmul(
                                rb_ps[:], ones_row[:], recip_r[:], start=True, stop=True
                            )
                            recipB = p2.tile([128, CH], F32, name="recipB", tag="recipB", bufs=1)
                            nc.scalar.activation(recipB[:], rb_ps[:], AF.Copy)
                            # attn^T = (P @ V)^T scaled by 1/denom, two 4-bank d passes
                            for half in range(2):
                                for d4 in range(4):
                                    d_ = half * 4 + d4
                                    aps = pspool.tile([128, CH], F32, name=f"at{d4}", tag=f"at{d4}", bufs=1)
                                    for k in range(nkt):
                                        nc.tensor.matmul(
                                            aps[:],
                                            vt[k][:, ts(d_, 128)],
                                            pt[k][:],
                                            start=(k == 0), stop=(k == nkt - 1),
                                        )
                                    nc.vector.tensor_tensor(
                                        out=attnT[d_][ch][:], in0=aps[:], in1=recipB[:],
                                        op=ALU.mult,
                                    )

            # ---------------- P3: output projection ----------------
            with tc.tile_pool(name="p34", bufs=1) as p34:
                outT = [
                    [p34.tile([128, CH], F32R, name=f"oT{dt_}_{c}", tag=f"oT{dt_}_{c}") for c in range(2)]
                    for dt_ in range(NDT)
                ]
                with tc.tile_pool(name="p3", bufs=1) as p3:
                    wo_sb = [p3.tile([128, D], F32R, name=f"wo{i}", tag=f"wo{i}") for i in range(NDT)]
                    for h in range(2):
                        for i in range(NDT):
                            nc.sync.dma_start(
                                wo_sb[i][:, ts(h, 512)], woT[ts(i, 128), ts(h, 512)]
                            )
                    for ch in range(2):
                        for ot in range(NDT):
                            ps = pspool.tile([128, CH], F32, name="mm", tag="mm", bufs=3)
                            for i in range(NDT):
                                nc.tensor.matmul(
                                    ps[:],
                                    wo_sb[i][:, ts(ot, 128)],
                                    attnT[i][ch][:],
                                    start=(i == 0), stop=(i == NDT - 1),
                                )
                            nc.scalar.activation(
                                outT[ot][ch][:], ps[:], AF.Identity, bias=bo2_sb[:, ot : ot + 1]
                            )

                # ---------------- P4: FFN + GELU ----------------
                with tc.tile_pool(name="p4", bufs=1) as p4:
                    for mb in range(M // 512):
                        wfb = [
                            p4.tile([128, 512], F32R, name=f"wf{i}", tag=f"wf{i}", bufs=2)
                            for i in range(NDT)
                        ]
                        for i in range(NDT):
                            nc.sync.dma_start(wfb[i][:], wfT[ts(i, 128), ts(mb, 512)])
                        for mt in range(4):
                            m = mb * 4 + mt
                            for ch in range(2):
                                ps = pspool.tile([128, CH], F32, name="mm", tag="mm", bufs=3)
                                for i in range(NDT):
                                    nc.tensor.matmul(
                                        ps[:],
                                        wfb[i][:, ts(mt, 128)],
                                        outT[i][ch][:],
                                        start=(i == 0), stop=(i == NDT - 1),
                                    )
                                st = p4.tile([128, CH], F32, name="ffstage", tag="ffstage", bufs=4)
                                nc.scalar.activation(
                                    st[:], ps[:], AF.Gelu, bias=bf_sb[:, m : m + 1]
                                )
                                nc.sync.dma_start(ffT[ts(m, 128), ts(ch, CH)], st[:])

    nc.compile()
    return nc


def _get_program():
    global _PROGRAM
    if _PROGRAM is None:
        _PROGRAM = _build_program()
    return _PROGRAM


def _owned_ranges(core):
    """(a0, b0): start rows of the two 512-token chunks core owns."""
    half = core % 2
    if half == 0:
        return 0, 3 * CH  # chunks 0, 3
    return CH, 2 * CH  # chunks 1, 2


def _make_in_maps(x, Wq, bq, Wk, bk, Wv, bv, Wo, bo, Wf, bf):
    f32 = np.float32
    wqT = np.ascontiguousarray(Wq.T, dtype=f32)
    wkT = np.ascontiguousarray(Wk.T, dtype=f32)
    wvT = np.ascontiguousarray(Wv.T, dtype=f32)
    woT = np.ascontiguousarray(Wo.T, dtype=f32)
    wfT = np.ascontiguousarray(Wf.T, dtype=f32)
    bo2 = (Wo.astype(np.float64) @ bv.astype(np.float64) + bo.astype(np.float64))
    bo2 = np.ascontiguousarray(bo2.astype(f32).reshape(D // 128, 128).T)
    bfT = np.ascontiguousarray(bf.reshape(M // 128, 128).T, dtype=f32)
    iota = (
        np.arange(128, dtype=f32)[:, None]
        + 128.0 * np.arange(S // 128, dtype=f32)[None, :]
    )
    shared = {
        "wqT": wqT, "wkT": wkT, "wvT": wvT, "woT": woT, "wfT": wfT,
        "bq": np.ascontiguousarray(bq.reshape(D // 128, 128).T, dtype=f32),
        "bk": np.ascontiguousarray(bk.reshape(D // 128, 128).T, dtype=f32),
        "bo2": bo2,
        "bfT": bfT,
        "iota_kt": np.ascontiguousarray(iota),
    }
    in_maps = []
    for core in range(N_CORES):
        b = core // 2
        a0, b0 = _owned_ranges(core)
        xTb = np.ascontiguousarray(x[b].T, dtype=f32)  # [D, S]
        xoT = np.ascontiguousarray(
            np.concatenate([xTb[:, a0 : a0 + CH], xTb[:, b0 : b0 + CH]], axis=1)
        )
        qp = np.concatenate(
            [np.arange(a0, a0 + CH), np.arange(b0, b0 + CH)]
        ).astype(f32)[None, :]
        in_maps.append(
            {**shared, "xT": xTb, "xoT": xoT, "qpos": np.ascontiguousarray(qp)}
        )
    return in_maps


def _run(inputs, trace=False, trace_cores=None, tmpdir=None):
    import sys

    if "/opt/trn_rl_repo" not in sys.path:
        sys.path.insert(0, "/opt/trn_rl_repo")
    from concourse.bass_utils import run_bass_kernel_spmd

    nc = _get_program()
    in_maps = _make_in_maps(**inputs)
    res = run_bass_kernel_spmd(
        nc, in_maps, list(range(N_CORES)), trace=trace, trace_cores=trace_cores,
        tmpdir=tmpdir,
    )
    out = np.empty((B, S, M), dtype=np.float32)
    for core in range(N_CORES):
        b = core // 2
        a0, b0 = _owned_ranges(core)
        ffT = res.results[core]["ffT"]  # [M, 1024]
        out[b, a0 : a0 + CH] = ffT[:, :CH].T
        out[b, b0 : b0 + CH] = ffT[:, CH:].T
    return out, res


def kernel(**inputs):
    out, _ = _run(inputs)
    return out



# revision 7
# speedup vs baseline: 1.4281x; 1.4281x over previous
"""Decoder block (single-head causal attention + GELU FFN) on 8 TRN2 NeuronCores.

Sharding: pure data parallel, no collectives. Core c handles batch b = c//2 and
1024 query tokens of that batch, chosen as two 512-token chunks that balance the
causal-attention workload:
  even cores (half 0): chunks 0 and 3  (rows    0:512  and 1536:2048)
  odd  cores (half 1): chunks 1 and 2  (rows  512:1024 and 1024:1536)
The SPMD program is identical on every core; all per-core differences are data.

Precision strategy (validated numerically against the fp64 reference):
  - All main matmuls run in fp8e4m3 with DoubleRow perf mode (2x128
    contraction per instruction at half the cycle cost): QKV projections,
    QK^T scores, probs@V, out projection, FFN. Host pre-scales x by 16 and
    all weights by 32; probs are scaled by 8 so exp() output fits fp8
    (max normal 240). PSUMs are fp32 throughout.
  - Causal rows with few visible keys lack error averaging, so the first 128
    slot-A query columns (global rows 0-127 on even cores, 512-639 on odd)
    are recomputed in an fp16 "patch" path end-to-end: fp16 projections of
    the first 128 tokens/queries, fp16 attention over the first 768 keys,
    fp16 out-proj + FFN, overwriting the fp8 result for those columns.
  - Output is written fp16 and upcast on host.

Layout: feature-major on chip; host packs every operand as [128, nslice*F]
so each tensor loads with one contiguous DMA. V stays resident in SBUF.
"""

import numpy as np
import ml_dtypes

D = 1024
S = 2048
B = 4
M = 4096
CH = 512
ND = 8  # 128-wide slices of D
NKT = [8, 16]  # key tiles per slot
PQ = 128  # patch query columns
PKT = 6  # patch key tiles (768 keys)
N_CORES = 8

F8NP = ml_dtypes.float8_e4m3
LN_C = float(np.log(8.0))  # probs pre-scale C=8 applied inside exp

_PROGRAM = None


def _build_program():
    import sys

    if "/opt/trn_rl_repo" not in sys.path:
        sys.path.insert(0, "/opt/trn_rl_repo")
    import concourse.bass as bass
    import concourse.tile as tile
    import concourse.mybir as mybir
    from concourse import bacc
    from concourse.bass import ts

    dt = mybir.dt
    AF = mybir.ActivationFunctionType
    ALU = mybir.AluOpType
    DR = mybir.MatmulPerfMode.DoubleRow
    F32, F16, F8, F32R = dt.float32, dt.float16, dt.float8e4, dt.float32r

    nc = bacc.Bacc("TRN2", target_bir_lowering=False, debug=False)

    # ---------------- DRAM I/O (all pre-packed host side) ----------------
    px8 = nc.dram_tensor("px8", [128, 4 * ND * 512], F8, kind="ExternalInput").ap()
    pxo8 = nc.dram_tensor("pxo8", [128, 2 * ND * 512], F8, kind="ExternalInput").ap()
    px16h = nc.dram_tensor("px16h", [128, ND * PQ], F16, kind="ExternalInput").ap()
    pxo16h = nc.dram_tensor("pxo16h", [128, ND * PQ], F16, kind="ExternalInput").ap()
    pwq8 = nc.dram_tensor("pwq8", [128, ND * D], F8, kind="ExternalInput").ap()
    pwk8 = nc.dram_tensor("pwk8", [128, ND * D], F8, kind="ExternalInput").ap()
    pwv8 = nc.dram_tensor("pwv8", [128, ND * D], F8, kind="ExternalInput").ap()
    pwo8 = nc.dram_tensor("pwo8", [128, ND * D], F8, kind="ExternalInput").ap()
    pwq16 = nc.dram_tensor("pwq16", [128, ND * D], F16, kind="ExternalInput").ap()
    pwk16 = nc.dram_tensor("pwk16", [128, ND * D], F16, kind="ExternalInput").ap()
    pwv16 = nc.dram_tensor("pwv16", [128, ND * D], F16, kind="ExternalInput").ap()
    pwo16 = nc.dram_tensor("pwo16", [128, ND * D], F16, kind="ExternalInput").ap()
    pwf8 = nc.dram_tensor("pwf8", [128, ND * M], F8, kind="ExternalInput").ap()
    pwf16 = nc.dram_tensor("pwf16", [128, ND * M], F16, kind="ExternalInput").ap()
    bq8 = nc.dram_tensor("bq8", [128, ND], F32, kind="ExternalInput").ap()
    bk8 = nc.dram_tensor("bk8", [128, ND], F32, kind="ExternalInput").ap()
    bq16 = nc.dram_tensor("bq16", [128, ND], F32, kind="ExternalInput").ap()
    bk16 = nc.dram_tensor("bk16", [128, ND], F32, kind="ExternalInput").ap()
    bo8 = nc.dram_tensor("bo8", [128, ND], F32, kind="ExternalInput").ap()
    bo16 = nc.dram_tensor("bo16", [128, ND], F32, kind="ExternalInput").ap()
    bf32 = nc.dram_tensor("bf32", [128, M // 128], F32, kind="ExternalInput").ap()
    qpos = nc.dram_tensor("qpos", [1, 2 * CH], F32R, kind="ExternalInput").ap()
    iota_kt = nc.dram_tensor("iota_kt", [128, S // 128], F32, kind="ExternalInput").ap()
    ffT16 = nc.dram_tensor("ffT16", [M, 2 * CH], F16, kind="ExternalOutput").ap()

    with tile.TileContext(nc) as tc:
        with (
            tc.tile_pool(name="const", bufs=1) as cpool,
            tc.tile_pool(name="persist", bufs=1) as ppool,
            tc.tile_pool(name="psum", bufs=1, space="PSUM") as pspool,
        ):
            # ---------------- constants ----------------
            ones_row_f = cpool.tile([1, 128], F32, name="ones_row_f", tag="ones_row_f")
            nc.vector.memset(ones_row_f[:], 1.0)
            ones_row = cpool.tile([1, 128], F32R, name="ones_row", tag="ones_row")
            nc.vector.tensor_copy(ones_row[:], ones_row_f[:])
            ones16 = cpool.tile([128, 1], F16, name="ones16", tag="ones16")
            nc.vector.memset(ones16[:], 1.0)
            ones8x2 = cpool.tile([128, 2, 16], F8, name="ones8x2", tag="ones8x2")
            nc.vector.memset(ones8x2[:], 1.0)
            lnc_col = cpool.tile([128, 1], F32, name="lnc", tag="lnc")
            nc.vector.memset(lnc_col[:], LN_C)
            iota_sb = cpool.tile([128, S // 128], F32, name="iota", tag="iota")
            nc.sync.dma_start(iota_sb[:], iota_kt[:])
            bq8_sb = cpool.tile([128, ND], F32, name="bq8", tag="bq8")
            nc.sync.dma_start(bq8_sb[:], bq8[:])
            bk8_sb = cpool.tile([128, ND], F32, name="bk8", tag="bk8")
            nc.sync.dma_start(bk8_sb[:], bk8[:])
            bq16_sb = cpool.tile([128, ND], F32, name="bq16", tag="bq16")
            nc.sync.dma_start(bq16_sb[:], bq16[:])
            bk16_sb = cpool.tile([128, ND], F32, name="bk16", tag="bk16")
            nc.sync.dma_start(bk16_sb[:], bk16[:])
            bo8_sb = cpool.tile([128, ND], F32, name="bo8", tag="bo8")
            nc.sync.dma_start(bo8_sb[:], bo8[:])
            bo16_sb = cpool.tile([128, ND], F32, name="bo16", tag="bo16")
            nc.sync.dma_start(bo16_sb[:], bo16[:])
            bf_sb = cpool.tile([128, M // 128], F32, name="bf", tag="bf")
            nc.sync.dma_start(bf_sb[:], bf32[:])
            qpos_row = cpool.tile([1, 2 * CH], F32R, name="qpos_row", tag="qpos_row")
            nc.sync.dma_start(qpos_row[:], qpos[:])

            # broadcast qpos to 128 partitions via ones outer-product
            qposB = cpool.tile([128, 2 * CH], F32, name="qposB", tag="qposB")
            for i in range(2):
                bc_ps = pspool.tile([128, 512], F32, name="small", tag="small", bufs=1)
                nc.tensor.matmul(
                    bc_ps[:], ones_row[:], qpos_row[:, ts(i, 512)], start=True, stop=True
                )
                nc.scalar.activation(qposB[:, ts(i, 512)], bc_ps[:], AF.Copy)

            # causal masks, precomputed once (gpsimd, off the critical path)
            # main: j 0..7 -> slotA ktile j; j 8..15 -> slotB ktile j
            msk8 = cpool.tile([128, 16, 512], F8, name="msk8", tag="msk8")
            for j in range(16):
                ch = 0 if j < 8 else 1
                nc.gpsimd.tensor_scalar(
                    out=msk8[:, j, :],
                    in0=qposB[:, ts(ch, CH)],
                    scalar1=iota_sb[:, j : j + 1],
                    scalar2=None,
                    op0=ALU.is_ge,
                )
            mskp = cpool.tile([128, PKT, PQ], F16, name="mskp", tag="mskp")
            for k in range(PKT):
                nc.gpsimd.tensor_scalar(
                    out=mskp[:, k, :],
                    in0=qposB[:, 0:PQ],
                    scalar1=iota_sb[:, k : k + 1],
                    scalar2=None,
                    op0=ALU.is_ge,
                )

            # ---------------- persistent attention tensors ----------------
            kT8 = ppool.tile([128, ND, S], F8, name="kT8", tag="kT8")
            K16 = ppool.tile([128, ND, PKT * 128], F16, name="K16", tag="K16")
            V8 = ppool.tile([128, 16, D], F8, name="V8", tag="V8")
            V16 = ppool.tile([128, PKT, D], F16, name="V16", tag="V16")
            qT8 = ppool.tile([128, ND, 2 * CH], F8, name="qT8", tag="qT8")
            Q16p = ppool.tile([128, ND, PQ], F16, name="Q16p", tag="Q16p")
            attnT8 = ppool.tile([128, ND, 2 * CH], F8, name="attnT8", tag="attnT8")
            attn16p = ppool.tile([128, ND, PQ], F16, name="attn16p", tag="attn16p")
            pt8a = ppool.tile([128, 8, 512], F8, name="pt8a", tag="pt8a")
            pt8b = ppool.tile([128, 16, 512], F8, name="pt8b", tag="pt8b")
            pt16 = ppool.tile([128, PKT, PQ], F16, name="pt16", tag="pt16")
            outT8 = ppool.tile([128, ND, 2 * CH], F8, name="outT8", tag="outT8")
            outT16p = ppool.tile([128, ND, PQ], F16, name="outT16p", tag="outT16p")

            # ---------------- P1: projections (all fp8 DoubleRow) ----------------
            with tc.tile_pool(name="p1", bufs=1) as p1:
                x8 = [p1.tile([128, ND, 512], F8, name=f"x8_{tb}", tag=f"x8_{tb}") for tb in range(4)]
                for tb in range(4):
                    nc.sync.dma_start(x8[tb][:], px8[:, ts(tb, ND * 512)])
                wk8_sb = p1.tile([128, ND, D], F8, name="wk8", tag="wk8")
                nc.sync.dma_start(wk8_sb[:], pwk8[:])
                wv8_sb = p1.tile([128, ND, D], F8, name="wv8", tag="wv8")
                nc.sync.dma_start(wv8_sb[:], pwv8[:])
                xo8 = [p1.tile([128, ND, 512], F8, name=f"xo8_{qb}", tag=f"xo8_{qb}") for qb in range(2)]
                for qb in range(2):
                    nc.sync.dma_start(xo8[qb][:], pxo8[:, ts(qb, ND * 512)])
                wq8_sb = p1.tile([128, ND, D], F8, name="wq8", tag="wq8")
                nc.sync.dma_start(wq8_sb[:], pwq8[:])

                # K pass: kT8 (+ fp16 casts of keys 128..767)
                for tb in range(4):
                    for ot in range(ND):
                        ps = pspool.tile([128, 512], F32, name="mm", tag="mm", bufs=3)
                        for i2 in range(4):
                            nc.tensor.matmul(
                                ps[:],
                                wk8_sb[:, 2 * i2 : 2 * i2 + 2, ts(ot, 128)],
                                x8[tb][:, 2 * i2 : 2 * i2 + 2, :],
                                start=(i2 == 0), stop=(i2 == 3), perf_mode=DR,
                            )
                        nc.scalar.activation(
                            kT8[:, ot, ts(tb, 512)], ps[:], AF.Identity,
                            bias=bk8_sb[:, ot : ot + 1], scale=1.0 / 16.0,
                        )
                        if tb == 0:
                            nc.scalar.activation(
                                K16[:, ot, 128:512], ps[:, 128:512], AF.Identity,
                                bias=bk16_sb[:, ot : ot + 1], scale=1.0 / 512.0,
                            )
                        elif tb == 1:
                            nc.scalar.activation(
                                K16[:, ot, 512:768], ps[:, 0:256], AF.Identity,
                                bias=bk16_sb[:, ot : ot + 1], scale=1.0 / 512.0,
                            )

                # V pass: token-major V8 (+ fp16 casts of tokens 128..767)
                for tb in range(4):
                    for tt in range(4):
                        kidx = tb * 4 + tt
                        for ob in range(2):
                            ps = pspool.tile([128, 512], F32, name="mm", tag="mm", bufs=3)
                            for i2 in range(4):
                                nc.tensor.matmul(
                                    ps[:],
                                    x8[tb][:, 2 * i2 : 2 * i2 + 2, ts(tt, 128)],
                                    wv8_sb[:, 2 * i2 : 2 * i2 + 2, ts(ob, 512)],
                                    start=(i2 == 0), stop=(i2 == 3), perf_mode=DR,
                                )
                            nc.scalar.activation(
                                V8[:, kidx, ts(ob, 512)], ps[:], AF.Copy, scale=1.0 / 16.0
                            )
                            if 1 <= kidx < PKT:
                                nc.scalar.activation(
                                    V16[:, kidx, ts(ob, 512)], ps[:], AF.Copy,
                                    scale=1.0 / 512.0,
                                )

                # Q pass (own tokens only)
                for qb in range(2):
                    for ot in range(ND):
                        ps = pspool.tile([128, 512], F32, name="mm", tag="mm", bufs=3)
                        for i2 in range(4):
                            nc.tensor.matmul(
                                ps[:],
                                wq8_sb[:, 2 * i2 : 2 * i2 + 2, ts(ot, 128)],
                                xo8[qb][:, 2 * i2 : 2 * i2 + 2, :],
                                start=(i2 == 0), stop=(i2 == 3), perf_mode=DR,
                            )
                        nc.scalar.activation(
                            qT8[:, ot, ts(qb, 512)], ps[:], AF.Identity,
                            bias=bq8_sb[:, ot : ot + 1], scale=1.0 / 16.0,
                        )

            def attention(ch, pt, tmp):
                nkt = NKT[ch]
                dn_ps = pspool.tile([1, 512], F32, name="dn", tag="dn", bufs=1)
                for k in range(nkt):
                    ps = pspool.tile([128, 512], F32, name="mm", tag="mm", bufs=3)
                    for i2 in range(4):
                        nc.tensor.matmul(
                            ps[:],
                            kT8[:, 2 * i2 : 2 * i2 + 2, ts(k, 128)],
                            qT8[:, 2 * i2 : 2 * i2 + 2, ts(ch, 512)],
                            start=(i2 == 0), stop=(i2 == 3), perf_mode=DR,
                        )
                    masked = (ch == 0) or (k >= 8)
                    if masked:
                        praw = tmp.tile([128, 512], F16, name="praw", tag="praw", bufs=3)
                        nc.scalar.activation(
                            praw[:], ps[:], AF.Exp, scale=1.0 / 32768.0, bias=lnc_col[:, 0:1]
                        )
                        nc.vector.tensor_tensor(
                            out=pt[:, k, :], in0=praw[:], in1=msk8[:, k, :], op=ALU.mult
                        )
                    else:
                        nc.scalar.activation(
                            pt[:, k, :], ps[:], AF.Exp, scale=1.0 / 32768.0, bias=lnc_col[:, 0:1]
                        )
                    if k % 2 == 1:
                        nc.tensor.matmul(
                            dn_ps[:], ones8x2[:, :, 0:1], pt[:, k - 1 : k + 1, :],
                            start=(k == 1), stop=(k == nkt - 1), perf_mode=DR,
                        )
                recip = tmp.tile([1, 512], F32, name="recip", tag="recip", bufs=2)
                nc.vector.reciprocal(recip[:], dn_ps[:])
                recip_r = tmp.tile([1, 512], F32R, name="recip_r", tag="recip_r", bufs=2)
                nc.vector.tensor_copy(recip_r[:], recip[:])
                rb_ps = pspool.tile([128, 512], F32, name="small", tag="small", bufs=1)
                nc.tensor.matmul(rb_ps[:], ones_row[:], recip_r[:], start=True, stop=True)
                recipB = tmp.tile([128, 512], F32, name="recipB", tag="recipB", bufs=2)
                nc.scalar.activation(recipB[:], rb_ps[:], AF.Copy)
                for d_ in range(ND):
                    aps = pspool.tile([128, 512], F32, name="mm", tag="mm", bufs=3)
                    for j2 in range(nkt // 2):
                        nc.tensor.matmul(
                            aps[:],
                            V8[:, 2 * j2 : 2 * j2 + 2, ts(d_, 128)],
                            pt[:, 2 * j2 : 2 * j2 + 2, :],
                            start=(j2 == 0), stop=(j2 == nkt // 2 - 1), perf_mode=DR,
                        )
                    nc.vector.tensor_tensor(
                        out=attnT8[:, d_, ts(ch, 512)], in0=aps[:], in1=recipB[:],
                        op=ALU.mult,
                    )

            def outproj(ch, wo8_sb):
                for ot in range(ND):
                    ps = pspool.tile([128, 512], F32, name="mm", tag="mm", bufs=3)
                    for i2 in range(4):
                        nc.tensor.matmul(
                            ps[:],
                            wo8_sb[:, 2 * i2 : 2 * i2 + 2, ts(ot, 128)],
                            attnT8[:, 2 * i2 : 2 * i2 + 2, ts(ch, 512)],
                            start=(i2 == 0), stop=(i2 == 3), perf_mode=DR,
                        )
                    nc.scalar.activation(
                        outT8[:, ot, ts(ch, 512)], ps[:], AF.Identity,
                        bias=bo8_sb[:, ot : ot + 1], scale=1.0 / 32.0,
                    )

            with tc.tile_pool(name="p2", bufs=1) as p2:
                wo8_sb = p2.tile([128, ND, D], F8, name="wo8", tag="wo8")
                nc.sync.dma_start(wo8_sb[:], pwo8[:])

                # slot A attention + outproj (fp8)
                attention(0, pt8a, p2)
                outproj(0, wo8_sb)

                # ---- fp16 patch: projections ----
                x16h = p2.tile([128, ND, PQ], F16, name="x16h", tag="x16h")
                nc.sync.dma_start(x16h[:], px16h[:])
                xo16h = p2.tile([128, ND, PQ], F16, name="xo16h", tag="xo16h")
                nc.sync.dma_start(xo16h[:], pxo16h[:])
                wk16_sb = p2.tile([128, ND, D], F16, name="w16", tag="w16", bufs=2)
                nc.sync.dma_start(wk16_sb[:], pwk16[:])
                wv16_sb = p2.tile([128, ND, D], F16, name="w16", tag="w16", bufs=2)
                nc.sync.dma_start(wv16_sb[:], pwv16[:])
                wq16_sb = p2.tile([128, ND, D], F16, name="w16", tag="w16", bufs=2)
                nc.sync.dma_start(wq16_sb[:], pwq16[:])
                wo16_sb = p2.tile([128, ND, D], F16, name="w16", tag="w16", bufs=2)
                nc.sync.dma_start(wo16_sb[:], pwo16[:])

                for ot in range(ND):
                    ps = pspool.tile([128, PQ], F32, name="pmm", tag="pmm", bufs=2)
                    for i in range(ND):
                        nc.tensor.matmul(
                            ps[:], wk16_sb[:, i, ts(ot, 128)], x16h[:, i, :],
                            start=(i == 0), stop=(i == ND - 1),
                        )
                    nc.scalar.activation(
                        K16[:, ot, 0:128], ps[:], AF.Identity,
                        bias=bk16_sb[:, ot : ot + 1],
                    )
                for ob in range(2):
                    ps = pspool.tile([128, 512], F32, name="mm", tag="mm", bufs=3)
                    for i in range(ND):
                        nc.tensor.matmul(
                            ps[:], x16h[:, i, :], wv16_sb[:, i, ts(ob, 512)],
                            start=(i == 0), stop=(i == ND - 1),
                        )
                    nc.scalar.activation(V16[:, 0, ts(ob, 512)], ps[:], AF.Copy)
                for ot in range(ND):
                    ps = pspool.tile([128, PQ], F32, name="pmm", tag="pmm", bufs=2)
                    for i in range(ND):
                        nc.tensor.matmul(
                            ps[:], wq16_sb[:, i, ts(ot, 128)], xo16h[:, i, :],
                            start=(i == 0), stop=(i == ND - 1),
                        )
                    nc.scalar.activation(
                        Q16p[:, ot, :], ps[:], AF.Identity,
                        bias=bq16_sb[:, ot : ot + 1],
                    )

                # ---- fp16 patch: attention over first 768 keys ----
                dnp = pspool.tile([1, PQ], F32, name="dn", tag="dn", bufs=1)
                for k in range(PKT):
                    ps = pspool.tile([128, PQ], F32, name="pmm", tag="pmm", bufs=2)
                    for i in range(ND):
                        nc.tensor.matmul(
                            ps[:], K16[:, i, ts(k, 128)], Q16p[:, i, :],
                            start=(i == 0), stop=(i == ND - 1),
                        )
                    prawp = p2.tile([128, PQ], F16, name="prawp", tag="prawp", bufs=3)
                    nc.scalar.activation(prawp[:], ps[:], AF.Exp, scale=1.0 / 32.0)
                    nc.vector.tensor_tensor(
                        out=pt16[:, k, :], in0=prawp[:], in1=mskp[:, k, :], op=ALU.mult
                    )
                    nc.tensor.matmul(
                        dnp[:], ones16[:], pt16[:, k, :],
                        start=(k == 0), stop=(k == PKT - 1),
                    )
                recp = p2.tile([1, PQ], F32, name="recp", tag="recp", bufs=2)
                nc.vector.reciprocal(recp[:], dnp[:])
                recp_r = p2.tile([1, PQ], F32R, name="recp_r", tag="recp_r", bufs=2)
                nc.vector.tensor_copy(recp_r[:], recp[:])
                rbp_ps = pspool.tile([128, PQ], F32, name="pmm", tag="pmm", bufs=2)
                nc.tensor.matmul(rbp_ps[:], ones_row[:], recp_r[:], start=True, stop=True)
                recipBp = p2.tile([128, PQ], F32, name="recipBp", tag="recipBp", bufs=1)
                nc.scalar.activation(recipBp[:], rbp_ps[:], AF.Copy)
                for d_ in range(ND):
                    aps = pspool.tile([128, PQ], F32, name="pmm", tag="pmm", bufs=2)
                    for j in range(PKT):
                        nc.tensor.matmul(
                            aps[:], V16[:, j, ts(d_, 128)], pt16[:, j, :],
                            start=(j == 0), stop=(j == PKT - 1),
                        )
                    nc.vector.tensor_tensor(
                        out=attn16p[:, d_, :], in0=aps[:], in1=recipBp[:], op=ALU.mult
                    )
                # patch outproj
                for ot in range(ND):
                    ps = pspool.tile([128, PQ], F32, name="pmm", tag="pmm", bufs=2)
                    for i in range(ND):
                        nc.tensor.matmul(
                            ps[:], wo16_sb[:, i, ts(ot, 128)], attn16p[:, i, :],
                            start=(i == 0), stop=(i == ND - 1),
                        )
                    nc.scalar.activation(
                        outT16p[:, ot, :], ps[:], AF.Identity,
                        bias=bo16_sb[:, ot : ot + 1],
                    )

                # slot B attention + outproj (fp8)
                attention(1, pt8b, p2)
                outproj(1, wo8_sb)

            # ---------------- P4: FFN + GELU (fp8 DR + fp16 patch) ----------------
            with tc.tile_pool(name="p4", bufs=1) as p4:
                for mb in range(M // 512):
                    wf8b = p4.tile([128, ND, 512], F8, name="wf8b", tag="wf8b", bufs=2)
                    nc.sync.dma_start(wf8b[:], pwf8[:, ts(mb, ND * 512)])
                    wf16b = p4.tile([128, ND, 512], F16, name="wf16b", tag="wf16b", bufs=2)
                    nc.sync.dma_start(wf16b[:], pwf16[:, ts(mb, ND * 512)])
                    for mt in range(4):
                        m = mb * 4 + mt
                        for ch in range(2):
                            ps = pspool.tile([128, 512], F32, name="mm", tag="mm", bufs=3)
                            for i2 in range(4):
                                nc.tensor.matmul(
                                    ps[:],
                                    wf8b[:, 2 * i2 : 2 * i2 + 2, ts(mt, 128)],
                                    outT8[:, 2 * i2 : 2 * i2 + 2, ts(ch, 512)],
                                    start=(i2 == 0), stop=(i2 == 3), perf_mode=DR,
                                )
                            st = p4.tile([128, 512], F16, name="ffstage", tag="ffstage", bufs=4)
                            nc.scalar.activation(
                                st[:], ps[:], AF.Gelu, scale=1.0 / 1024.0,
                                bias=bf_sb[:, m : m + 1],
                            )
                            if ch == 0:
                                pps = pspool.tile([128, PQ], F32, name="pmm", tag="pmm", bufs=2)
                                for i in range(ND):
                                    nc.tensor.matmul(
                                        pps[:], wf16b[:, i, ts(mt, 128)], outT16p[:, i, :],
                                        start=(i == 0), stop=(i == ND - 1),
                                    )
                                nc.scalar.activation(
                                    st[:, 0:128], pps[:], AF.Gelu,
                                    bias=bf_sb[:, m : m + 1],
                                )
                            nc.sync.dma_start(ffT16[ts(m, 128), ts(ch, 512)], st[:])

    nc.compile()
    return nc


def _get_program():
    global _PROGRAM
    if _PROGRAM is None:
        _PROGRAM = _build_program()
    return _PROGRAM


def _owned_ranges(core):
    """(a0, b0): start rows of the two 512-token chunks core owns."""
    half = core % 2
    if half == 0:
        return 0, 3 * CH  # chunks 0, 3
    return CH, 2 * CH  # chunks 1, 2


def _pack(matT, nsl=ND):
    """[nsl*128, F] -> [128, nsl*F] so tile [128, nsl, F] loads in one DMA."""
    Dd, F = matT.shape
    assert Dd == nsl * 128
    return np.ascontiguousarray(
        matT.reshape(nsl, 128, F).transpose(1, 0, 2).reshape(128, nsl * F)
    )


def _q8(a, s):
    return (np.asarray(a, np.float32) * np.float32(s)).astype(F8NP)


def _make_in_maps(x, Wq, bq, Wk, bk, Wv, bv, Wo, bo, Wf, bf):
    f32 = np.float32
    f16 = np.float16
    bo2 = (Wo.astype(np.float64) @ bv.astype(np.float64) + bo.astype(np.float64)).astype(f32)

    def bias_col(v, scale=1.0):
        return np.ascontiguousarray((np.asarray(v, f32) * f32(scale)).reshape(-1, 128).T)

    wq8m = _q8(Wq.T, 32.0)
    wk8m = _q8(Wk.T, 32.0)
    wv8m = _q8(Wv.T, 32.0)
    wo8m = _q8(Wo.T, 32.0)
    wf8m = _q8(Wf.T, 32.0)  # [1024, 4096]
    iota = (
        np.arange(128, dtype=f32)[:, None]
        + 128.0 * np.arange(S // 128, dtype=f32)[None, :]
    )

    def pack_mb(mat):  # [1024, 4096] -> [128, 8*4096] mb-major
        return np.ascontiguousarray(
            mat.reshape(ND, 128, 8, 512).transpose(1, 2, 0, 3).reshape(128, ND * M)
        )

    shared = {
        "pwq8": _pack(wq8m), "pwk8": _pack(wk8m), "pwv8": _pack(wv8m),
        "pwo8": _pack(wo8m),
        "pwq16": _pack(Wq.T.astype(f16)), "pwk16": _pack(Wk.T.astype(f16)),
        "pwv16": _pack(Wv.T.astype(f16)), "pwo16": _pack(Wo.T.astype(f16)),
        "pwf8": pack_mb(wf8m), "pwf16": pack_mb(Wf.T.astype(f16)),
        "bq8": bias_col(bq, 32.0), "bk8": bias_col(bk, 32.0),
        "bq16": bias_col(bq), "bk16": bias_col(bk),
        "bo8": bias_col(bo2, 32.0), "bo16": bias_col(bo2),
        "bf32": bias_col(bf),
        "iota_kt": np.ascontiguousarray(iota),
    }
    in_maps = []
    for core in range(N_CORES):
        b = core // 2
        a0, b0 = _owned_ranges(core)
        xT = np.ascontiguousarray(x[b].T.astype(f32))  # [D, S]
        x8m = _q8(xT, 16.0)  # [1024, 2048]
        # tb-major pack: [128, (tb, i, t)]
        px8v = np.ascontiguousarray(
            x8m.reshape(ND, 128, 4, 512).transpose(1, 2, 0, 3).reshape(128, 4 * ND * 512)
        )
        xo8m = _q8(
            np.concatenate([xT[:, a0 : a0 + CH], xT[:, b0 : b0 + CH]], axis=1), 16.0
        )
        pxo8v = np.ascontiguousarray(
            xo8m.reshape(ND, 128, 2, 512).transpose(1, 2, 0, 3).reshape(128, 2 * ND * 512)
        )
        px16hv = _pack(xT[:, 0:PQ].astype(f16))
        pxo16hv = _pack(xT[:, a0 : a0 + PQ].astype(f16))
        qp = np.concatenate(
            [np.arange(a0, a0 + CH), np.arange(b0, b0 + CH)]
        ).astype(f32)[None, :]
        in_maps.append(
            {
                **shared,
                "px8": px8v, "pxo8": pxo8v,
                "px16h": px16hv, "pxo16h": pxo16hv,
                "qpos": np.ascontiguousarray(qp),
            }
        )
    return in_maps


def _run(inputs, trace=False, trace_cores=None, tmpdir=None):
    import sys

    if "/opt/trn_rl_repo" not in sys.path:
        sys.path.insert(0, "/opt/trn_rl_repo")
    from concourse.bass_utils import run_bass_kernel_spmd

    nc = _get_program()
    in_maps = _make_in_maps(**inputs)
    res = run_bass_kernel_spmd(
        nc, in_maps, list(range(N_CORES)), trace=trace, trace_cores=trace_cores,
        tmpdir=tmpdir,
    )
    out = np.empty((B, S, M), dtype=np.float32)
    for core in range(N_CORES):
        b = core // 2
        a0, b0 = _owned_ranges(core)
        ffT = np.asarray(res.results[core]["ffT16"], dtype=np.float32)  # [M, 1024]
        out[b, a0 : a0 + CH] = ffT[:, :CH].T
        out[b, b0 : b0 + CH] = ffT[:, CH:].T
    return out, res


def kernel(**inputs):
    out, _ = _run(inputs)
    return out


# revision 8
# speedup vs baseline: 1.4922x; 1.0448x over previous
"""Decoder block (single-head causal attention + GELU FFN) on 8 TRN2 NeuronCores.

Sharding: pure data parallel, no collectives. Core c handles batch b = c//2 and
1024 query tokens of that batch, chosen as two 512-token chunks that balance the
causal-attention workload:
  even cores (half 0): chunks 0 and 3  (rows    0:512  and 1536:2048)
  odd  cores (half 1): chunks 1 and 2  (rows  512:1024 and 1024:1536)
The SPMD program is identical on every core; all per-core differences are data.

Precision strategy (validated numerically against the fp64 reference):
  - All main matmuls run in fp8e4m3 with DoubleRow perf mode (2x128
    contraction per instruction at half the cycle cost): QKV projections,
    QK^T scores, probs@V, out projection, FFN. Host pre-scales x by 16 and
    all weights by 32; probs are scaled by 8 so exp() output fits fp8
    (max normal 240). PSUMs are fp32 throughout.
  - Causal rows with few visible keys lack error averaging, so the first 128
    slot-A query columns (global rows 0-127 on even cores, 512-639 on odd)
    are recomputed in an fp16 "patch" path end-to-end: fp16 projections of
    the first 128 tokens/queries, fp16 attention over the first 768 keys,
    fp16 out-proj + FFN, overwriting the fp8 result for those columns.
  - Output is written fp16 and upcast on host.

Layout: feature-major on chip; host packs every operand as [128, nslice*F]
so each tensor loads with one contiguous DMA. V stays resident in SBUF.
"""

import numpy as np
import ml_dtypes

D = 1024
S = 2048
B = 4
M = 4096
CH = 512
ND = 8  # 128-wide slices of D
NKT = [8, 16]  # key tiles per slot
PQ = 128  # patch query columns
PKT = 6  # patch key tiles (768 keys)
N_CORES = 8

F8NP = ml_dtypes.float8_e4m3
LN_C = float(np.log(8.0))  # probs pre-scale C=8 applied inside exp

_PROGRAM = None


def _build_program():
    import sys

    if "/opt/trn_rl_repo" not in sys.path:
        sys.path.insert(0, "/opt/trn_rl_repo")
    import concourse.bass as bass
    import concourse.tile as tile
    import concourse.mybir as mybir
    from concourse import bacc
    from concourse.bass import ts

    dt = mybir.dt
    AF = mybir.ActivationFunctionType
    ALU = mybir.AluOpType
    DR = mybir.MatmulPerfMode.DoubleRow
    F32, F16, F8, F32R = dt.float32, dt.float16, dt.float8e4, dt.float32r

    nc = bacc.Bacc("TRN2", target_bir_lowering=False, debug=False)

    # ---------------- DRAM I/O (all pre-packed host side) ----------------
    px8 = nc.dram_tensor("px8", [128, 4 * ND * 512], F8, kind="ExternalInput").ap()
    pxo8 = nc.dram_tensor("pxo8", [128, 2 * ND * 512], F8, kind="ExternalInput").ap()
    px16h = nc.dram_tensor("px16h", [128, ND * PQ], F16, kind="ExternalInput").ap()
    pxo16h = nc.dram_tensor("pxo16h", [128, ND * PQ], F16, kind="ExternalInput").ap()
    pwq8 = nc.dram_tensor("pwq8", [128, ND * D], F8, kind="ExternalInput").ap()
    pwk8 = nc.dram_tensor("pwk8", [128, ND * D], F8, kind="ExternalInput").ap()
    pwv8 = nc.dram_tensor("pwv8", [128, ND * D], F8, kind="ExternalInput").ap()
    pwo8 = nc.dram_tensor("pwo8", [128, ND * D], F8, kind="ExternalInput").ap()
    pwq16 = nc.dram_tensor("pwq16", [128, ND * D], F16, kind="ExternalInput").ap()
    pwk16 = nc.dram_tensor("pwk16", [128, ND * D], F16, kind="ExternalInput").ap()
    pwv16 = nc.dram_tensor("pwv16", [128, ND * D], F16, kind="ExternalInput").ap()
    pwo16 = nc.dram_tensor("pwo16", [128, ND * D], F16, kind="ExternalInput").ap()
    pwf8 = nc.dram_tensor("pwf8", [128, ND * M], F8, kind="ExternalInput").ap()
    pwf16 = nc.dram_tensor("pwf16", [128, ND * M], F16, kind="ExternalInput").ap()
    bq8 = nc.dram_tensor("bq8", [128, ND], F32, kind="ExternalInput").ap()
    bk8 = nc.dram_tensor("bk8", [128, ND], F32, kind="ExternalInput").ap()
    bq16 = nc.dram_tensor("bq16", [128, ND], F32, kind="ExternalInput").ap()
    bk16 = nc.dram_tensor("bk16", [128, ND], F32, kind="ExternalInput").ap()
    bo8 = nc.dram_tensor("bo8", [128, ND], F32, kind="ExternalInput").ap()
    bo16 = nc.dram_tensor("bo16", [128, ND], F32, kind="ExternalInput").ap()
    bf32 = nc.dram_tensor("bf32", [128, M // 128], F32, kind="ExternalInput").ap()
    qpos = nc.dram_tensor("qpos", [1, 2 * CH], F32R, kind="ExternalInput").ap()
    iota_kt = nc.dram_tensor("iota_kt", [128, S // 128], F32, kind="ExternalInput").ap()
    ffT16 = nc.dram_tensor("ffT16", [M, 2 * CH], F16, kind="ExternalOutput").ap()

    with tile.TileContext(nc) as tc:
        with (
            tc.tile_pool(name="const", bufs=1) as cpool,
            tc.tile_pool(name="persist", bufs=1) as ppool,
            tc.tile_pool(name="psum", bufs=1, space="PSUM") as pspool,
        ):
            # ---------------- constants ----------------
            ones_row_f = cpool.tile([1, 128], F32, name="ones_row_f", tag="ones_row_f")
            nc.vector.memset(ones_row_f[:], 1.0)
            ones_row = cpool.tile([1, 128], F32R, name="ones_row", tag="ones_row")
            nc.vector.tensor_copy(ones_row[:], ones_row_f[:])
            ones16 = cpool.tile([128, 1], F16, name="ones16", tag="ones16")
            nc.vector.memset(ones16[:], 1.0)
            ones8x2 = cpool.tile([128, 2, 16], F8, name="ones8x2", tag="ones8x2")
            nc.vector.memset(ones8x2[:], 1.0)
            lnc_col = cpool.tile([128, 1], F32, name="lnc", tag="lnc")
            nc.vector.memset(lnc_col[:], LN_C)
            iota_sb = cpool.tile([128, S // 128], F32, name="iota", tag="iota")
            nc.sync.dma_start(iota_sb[:], iota_kt[:])
            bq8_sb = cpool.tile([128, ND], F32, name="bq8", tag="bq8")
            nc.sync.dma_start(bq8_sb[:], bq8[:])
            bk8_sb = cpool.tile([128, ND], F32, name="bk8", tag="bk8")
            nc.sync.dma_start(bk8_sb[:], bk8[:])
            bq16_sb = cpool.tile([128, ND], F32, name="bq16", tag="bq16")
            nc.sync.dma_start(bq16_sb[:], bq16[:])
            bk16_sb = cpool.tile([128, ND], F32, name="bk16", tag="bk16")
            nc.sync.dma_start(bk16_sb[:], bk16[:])
            bo8_sb = cpool.tile([128, ND], F32, name="bo8", tag="bo8")
            nc.sync.dma_start(bo8_sb[:], bo8[:])
            bo16_sb = cpool.tile([128, ND], F32, name="bo16", tag="bo16")
            nc.sync.dma_start(bo16_sb[:], bo16[:])
            bf_sb = cpool.tile([128, M // 128], F32, name="bf", tag="bf")
            nc.sync.dma_start(bf_sb[:], bf32[:])
            qpos_row = cpool.tile([1, 2 * CH], F32R, name="qpos_row", tag="qpos_row")
            nc.sync.dma_start(qpos_row[:], qpos[:])

            # broadcast qpos to 128 partitions via ones outer-product
            qposB = cpool.tile([128, 2 * CH], F32, name="qposB", tag="qposB")
            for i in range(2):
                bc_ps = pspool.tile([128, 512], F32, name="small", tag="small", bufs=1)
                nc.tensor.matmul(
                    bc_ps[:], ones_row[:], qpos_row[:, ts(i, 512)], start=True, stop=True
                )
                nc.scalar.activation(qposB[:, ts(i, 512)], bc_ps[:], AF.Copy)

            # causal masks, precomputed once (gpsimd, off the critical path)
            # main: j 0..7 -> slotA ktile j; j 8..15 -> slotB ktile j
            msk8 = cpool.tile([128, 16, 512], F8, name="msk8", tag="msk8")
            for j in range(16):
                ch = 0 if j < 8 else 1
                nc.vector.tensor_scalar(
                    out=msk8[:, j, :],
                    in0=qposB[:, ts(ch, CH)],
                    scalar1=iota_sb[:, j : j + 1],
                    scalar2=None,
                    op0=ALU.is_ge,
                )
            mskp = cpool.tile([128, PKT, PQ], F16, name="mskp", tag="mskp")
            for k in range(PKT):
                nc.vector.tensor_scalar(
                    out=mskp[:, k, :],
                    in0=qposB[:, 0:PQ],
                    scalar1=iota_sb[:, k : k + 1],
                    scalar2=None,
                    op0=ALU.is_ge,
                )

            # ---------------- persistent attention tensors ----------------
            kT8 = ppool.tile([128, ND, S], F8, name="kT8", tag="kT8")
            K16 = ppool.tile([128, ND, PKT * 128], F16, name="K16", tag="K16")
            V8 = ppool.tile([128, 16, D], F8, name="V8", tag="V8")
            V16 = ppool.tile([128, PKT, D], F16, name="V16", tag="V16")
            qT8 = ppool.tile([128, ND, 2 * CH], F8, name="qT8", tag="qT8")
            Q16p = ppool.tile([128, ND, PQ], F16, name="Q16p", tag="Q16p")
            attnT8 = ppool.tile([128, ND, 2 * CH], F8, name="attnT8", tag="attnT8")
            attn16p = ppool.tile([128, ND, PQ], F16, name="attn16p", tag="attn16p")
            pt8a = ppool.tile([128, 8, 512], F8, name="pt8a", tag="pt8a")
            pt8b = ppool.tile([128, 16, 512], F8, name="pt8b", tag="pt8b")
            pt16 = ppool.tile([128, PKT, PQ], F16, name="pt16", tag="pt16")
            outT8 = ppool.tile([128, ND, 2 * CH], F8, name="outT8", tag="outT8")
            outT16p = ppool.tile([128, ND, PQ], F16, name="outT16p", tag="outT16p")

            # ---------------- P1: projections (all fp8 DoubleRow) ----------------
            with tc.tile_pool(name="p1", bufs=1) as p1:
                x8 = [p1.tile([128, ND, 512], F8, name=f"x8_{tb}", tag=f"x8_{tb}") for tb in range(4)]
                for tb in range(4):
                    nc.sync.dma_start(x8[tb][:], px8[:, ts(tb, ND * 512)])
                wk8_sb = p1.tile([128, ND, D], F8, name="wk8", tag="wk8")
                nc.sync.dma_start(wk8_sb[:], pwk8[:])
                wv8_sb = p1.tile([128, ND, D], F8, name="wv8", tag="wv8")
                nc.sync.dma_start(wv8_sb[:], pwv8[:])
                xo8 = [p1.tile([128, ND, 512], F8, name=f"xo8_{qb}", tag=f"xo8_{qb}") for qb in range(2)]
                for qb in range(2):
                    nc.sync.dma_start(xo8[qb][:], pxo8[:, ts(qb, ND * 512)])
                wq8_sb = p1.tile([128, ND, D], F8, name="wq8", tag="wq8")
                nc.sync.dma_start(wq8_sb[:], pwq8[:])

                # K pass: kT8 (+ fp16 casts of keys 128..767)
                for tb in range(4):
                    for ot in range(ND):
                        ps = pspool.tile([128, 512], F32, name="mm", tag="mm", bufs=3)
                        for i2 in range(4):
                            nc.tensor.matmul(
                                ps[:],
                                wk8_sb[:, 2 * i2 : 2 * i2 + 2, ts(ot, 128)],
                                x8[tb][:, 2 * i2 : 2 * i2 + 2, :],
                                start=(i2 == 0), stop=(i2 == 3), perf_mode=DR,
                            )
                        nc.scalar.activation(
                            kT8[:, ot, ts(tb, 512)], ps[:], AF.Identity,
                            bias=bk8_sb[:, ot : ot + 1], scale=1.0 / 16.0,
                        )
                        if tb == 0:
                            nc.scalar.activation(
                                K16[:, ot, 128:512], ps[:, 128:512], AF.Identity,
                                bias=bk16_sb[:, ot : ot + 1], scale=1.0 / 512.0,
                            )
                        elif tb == 1:
                            nc.scalar.activation(
                                K16[:, ot, 512:768], ps[:, 0:256], AF.Identity,
                                bias=bk16_sb[:, ot : ot + 1], scale=1.0 / 512.0,
                            )

                # V pass: token-major V8 (+ fp16 casts of tokens 128..767)
                for tb in range(4):
                    for tt in range(4):
                        kidx = tb * 4 + tt
                        for ob in range(2):
                            ps = pspool.tile([128, 512], F32, name="mm", tag="mm", bufs=3)
                            for i2 in range(4):
                                nc.tensor.matmul(
                                    ps[:],
                                    x8[tb][:, 2 * i2 : 2 * i2 + 2, ts(tt, 128)],
                                    wv8_sb[:, 2 * i2 : 2 * i2 + 2, ts(ob, 512)],
                                    start=(i2 == 0), stop=(i2 == 3), perf_mode=DR,
                                )
                            nc.scalar.activation(
                                V8[:, kidx, ts(ob, 512)], ps[:], AF.Copy, scale=1.0 / 16.0
                            )
                            if 1 <= kidx < PKT:
                                nc.scalar.activation(
                                    V16[:, kidx, ts(ob, 512)], ps[:], AF.Copy,
                                    scale=1.0 / 512.0,
                                )

                # Q pass (own tokens only)
                for qb in range(2):
                    for ot in range(ND):
                        ps = pspool.tile([128, 512], F32, name="mm", tag="mm", bufs=3)
                        for i2 in range(4):
                            nc.tensor.matmul(
                                ps[:],
                                wq8_sb[:, 2 * i2 : 2 * i2 + 2, ts(ot, 128)],
                                xo8[qb][:, 2 * i2 : 2 * i2 + 2, :],
                                start=(i2 == 0), stop=(i2 == 3), perf_mode=DR,
                            )
                        nc.scalar.activation(
                            qT8[:, ot, ts(qb, 512)], ps[:], AF.Identity,
                            bias=bq8_sb[:, ot : ot + 1], scale=1.0 / 16.0,
                        )

            def attention(ch, pt, tmp):
                nkt = NKT[ch]
                dn_ps = pspool.tile([1, 512], F32, name="dn", tag="dn", bufs=1)
                for k in range(nkt):
                    ps = pspool.tile([128, 512], F32, name="mm", tag="mm", bufs=3)
                    for i2 in range(4):
                        nc.tensor.matmul(
                            ps[:],
                            kT8[:, 2 * i2 : 2 * i2 + 2, ts(k, 128)],
                            qT8[:, 2 * i2 : 2 * i2 + 2, ts(ch, 512)],
                            start=(i2 == 0), stop=(i2 == 3), perf_mode=DR,
                        )
                    masked = (ch == 0) or (k >= 8)
                    if masked:
                        praw = tmp.tile([128, 512], F16, name="praw", tag="praw", bufs=3)
                        nc.scalar.activation(
                            praw[:], ps[:], AF.Exp, scale=1.0 / 32768.0, bias=lnc_col[:, 0:1]
                        )
                        nc.vector.tensor_tensor(
                            out=pt[:, k, :], in0=praw[:], in1=msk8[:, k, :], op=ALU.mult
                        )
                    else:
                        nc.scalar.activation(
                            pt[:, k, :], ps[:], AF.Exp, scale=1.0 / 32768.0, bias=lnc_col[:, 0:1]
                        )
                    if k % 2 == 1:
                        nc.tensor.matmul(
                            dn_ps[:], ones8x2[:, :, 0:1], pt[:, k - 1 : k + 1, :],
                            start=(k == 1), stop=(k == nkt - 1), perf_mode=DR,
                        )
                recip = tmp.tile([1, 512], F32, name="recip", tag="recip", bufs=2)
                nc.vector.reciprocal(recip[:], dn_ps[:])
                recip_r = tmp.tile([1, 512], F32R, name="recip_r", tag="recip_r", bufs=2)
                nc.vector.tensor_copy(recip_r[:], recip[:])
                rb_ps = pspool.tile([128, 512], F32, name="small", tag="small", bufs=1)
                nc.tensor.matmul(rb_ps[:], ones_row[:], recip_r[:], start=True, stop=True)
                recipB = tmp.tile([128, 512], F32, name="recipB", tag="recipB", bufs=2)
                nc.scalar.activation(recipB[:], rb_ps[:], AF.Copy)
                for d_ in range(ND):
                    aps = pspool.tile([128, 512], F32, name="mm", tag="mm", bufs=3)
                    for j2 in range(nkt // 2):
                        nc.tensor.matmul(
                            aps[:],
                            V8[:, 2 * j2 : 2 * j2 + 2, ts(d_, 128)],
                            pt[:, 2 * j2 : 2 * j2 + 2, :],
                            start=(j2 == 0), stop=(j2 == nkt // 2 - 1), perf_mode=DR,
                        )
                    nc.vector.tensor_tensor(
                        out=attnT8[:, d_, ts(ch, 512)], in0=aps[:], in1=recipB[:],
                        op=ALU.mult,
                    )

            def outproj(ch, wo8_sb):
                for ot in range(ND):
                    ps = pspool.tile([128, 512], F32, name="mm", tag="mm", bufs=3)
                    for i2 in range(4):
                        nc.tensor.matmul(
                            ps[:],
                            wo8_sb[:, 2 * i2 : 2 * i2 + 2, ts(ot, 128)],
                            attnT8[:, 2 * i2 : 2 * i2 + 2, ts(ch, 512)],
                            start=(i2 == 0), stop=(i2 == 3), perf_mode=DR,
                        )
                    nc.scalar.activation(
                        outT8[:, ot, ts(ch, 512)], ps[:], AF.Identity,
                        bias=bo8_sb[:, ot : ot + 1], scale=1.0 / 32.0,
                    )

            with tc.tile_pool(name="p2", bufs=1) as p2:
                wo8_sb = p2.tile([128, ND, D], F8, name="wo8", tag="wo8")
                nc.sync.dma_start(wo8_sb[:], pwo8[:])

                # slot A attention + outproj (fp8)
                attention(0, pt8a, p2)
                outproj(0, wo8_sb)

                # ---- fp16 patch: projections ----
                x16h = p2.tile([128, ND, PQ], F16, name="x16h", tag="x16h")
                nc.sync.dma_start(x16h[:], px16h[:])
                xo16h = p2.tile([128, ND, PQ], F16, name="xo16h", tag="xo16h")
                nc.sync.dma_start(xo16h[:], pxo16h[:])
                wk16_sb = p2.tile([128, ND, D], F16, name="w16", tag="w16", bufs=2)
                nc.sync.dma_start(wk16_sb[:], pwk16[:])
                wv16_sb = p2.tile([128, ND, D], F16, name="w16", tag="w16", bufs=2)
                nc.sync.dma_start(wv16_sb[:], pwv16[:])
                wq16_sb = p2.tile([128, ND, D], F16, name="w16", tag="w16", bufs=2)
                nc.sync.dma_start(wq16_sb[:], pwq16[:])
                wo16_sb = p2.tile([128, ND, D], F16, name="w16", tag="w16", bufs=2)
                nc.sync.dma_start(wo16_sb[:], pwo16[:])

                for ot in range(ND):
                    ps = pspool.tile([128, PQ], F32, name="pmm", tag="pmm", bufs=2)
                    for i in range(ND):
                        nc.tensor.matmul(
                            ps[:], wk16_sb[:, i, ts(ot, 128)], x16h[:, i, :],
                            start=(i == 0), stop=(i == ND - 1),
                        )
                    nc.scalar.activation(
                        K16[:, ot, 0:128], ps[:], AF.Identity,
                        bias=bk16_sb[:, ot : ot + 1],
                    )
                for ob in range(2):
                    ps = pspool.tile([128, 512], F32, name="mm", tag="mm", bufs=3)
                    for i in range(ND):
                        nc.tensor.matmul(
                            ps[:], x16h[:, i, :], wv16_sb[:, i, ts(ob, 512)],
                            start=(i == 0), stop=(i == ND - 1),
                        )
                    nc.scalar.activation(V16[:, 0, ts(ob, 512)], ps[:], AF.Copy)
                for ot in range(ND):
                    ps = pspool.tile([128, PQ], F32, name="pmm", tag="pmm", bufs=2)
                    for i in range(ND):
                        nc.tensor.matmul(
                            ps[:], wq16_sb[:, i, ts(ot, 128)], xo16h[:, i, :],
                            start=(i == 0), stop=(i == ND - 1),
                        )
                    nc.scalar.activation(
                        Q16p[:, ot, :], ps[:], AF.Identity,
                        bias=bq16_sb[:, ot : ot + 1],
                    )

                # ---- fp16 patch: attention over first 768 keys ----
                dnp = pspool.tile([1, PQ], F32, name="dn", tag="dn", bufs=1)
                for k in range(PKT):
                    ps = pspool.tile([128, PQ], F32, name="pmm", tag="pmm", bufs=2)
                    for i in range(ND):
                        nc.tensor.matmul(
                            ps[:], K16[:, i, ts(k, 128)], Q16p[:, i, :],
                            start=(i == 0), stop=(i == ND - 1),
                        )
                    prawp = p2.tile([128, PQ], F16, name="prawp", tag="prawp", bufs=3)
                    nc.scalar.activation(prawp[:], ps[:], AF.Exp, scale=1.0 / 32.0)
                    nc.vector.tensor_tensor(
                        out=pt16[:, k, :], in0=prawp[:], in1=mskp[:, k, :], op=ALU.mult
                    )
                    nc.tensor.matmul(
                        dnp[:], ones16[:], pt16[:, k, :],
                        start=(k == 0), stop=(k == PKT - 1),
                    )
                recp = p2.tile([1, PQ], F32, name="recp", tag="recp", bufs=2)
                nc.vector.reciprocal(recp[:], dnp[:])
                recp_r = p2.tile([1, PQ], F32R, name="recp_r", tag="recp_r", bufs=2)
                nc.vector.tensor_copy(recp_r[:], recp[:])
                rbp_ps = pspool.tile([128, PQ], F32, name="pmm", tag="pmm", bufs=2)
                nc.tensor.matmul(rbp_ps[:], ones_row[:], recp_r[:], start=True, stop=True)
                recipBp = p2.tile([128, PQ], F32, name="recipBp", tag="recipBp", bufs=1)
                nc.scalar.activation(recipBp[:], rbp_ps[:], AF.Copy)
                for d_ in range(ND):
                    aps = pspool.tile([128, PQ], F32, name="pmm", tag="pmm", bufs=2)
                    for j in range(PKT):
                        nc.tensor.matmul(
                            aps[:], V16[:, j, ts(d_, 128)], pt16[:, j, :],
                            start=(j == 0), stop=(j == PKT - 1),
                        )
                    nc.vector.tensor_tensor(
                        out=attn16p[:, d_, :], in0=aps[:], in1=recipBp[:], op=ALU.mult
                    )
                # patch outproj
                for ot in range(ND):
                    ps = pspool.tile([128, PQ], F32, name="pmm", tag="pmm", bufs=2)
                    for i in range(ND):
                        nc.tensor.matmul(
                            ps[:], wo16_sb[:, i, ts(ot, 128)], attn16p[:, i, :],
                            start=(i == 0), stop=(i == ND - 1),
                        )
                    nc.scalar.activation(
                        outT16p[:, ot, :], ps[:], AF.Identity,
                        bias=bo16_sb[:, ot : ot + 1],
                    )

                # slot B attention + outproj (fp8)
                attention(1, pt8b, p2)
                outproj(1, wo8_sb)

            # ---------------- P4: FFN + GELU (fp8 DR + fp16 patch) ----------------
            with tc.tile_pool(name="p4", bufs=1) as p4:
                for mb in range(M // 512):
                    wf8b = p4.tile([128, ND, 512], F8, name="wf8b", tag="wf8b", bufs=2)
                    nc.sync.dma_start(wf8b[:], pwf8[:, ts(mb, ND * 512)])
                    wf16b = p4.tile([128, ND, 512], F16, name="wf16b", tag="wf16b", bufs=2)
                    nc.sync.dma_start(wf16b[:], pwf16[:, ts(mb, ND * 512)])
                    for mt in range(4):
                        m = mb * 4 + mt
                        for ch in range(2):
                            ps = pspool.tile([128, 512], F32, name="mm", tag="mm", bufs=3)
                            for i2 in range(4):
                                nc.tensor.matmul(
                                    ps[:],
                                    wf8b[:, 2 * i2 : 2 * i2 + 2, ts(mt, 128)],
                                    outT8[:, 2 * i2 : 2 * i2 + 2, ts(ch, 512)],
                                    start=(i2 == 0), stop=(i2 == 3), perf_mode=DR,
                                )
                            st = p4.tile([128, 512], F16, name="ffstage", tag="ffstage", bufs=4)
                            nc.scalar.activation(
                                st[:], ps[:], AF.Gelu, scale=1.0 / 1024.0,
                                bias=bf_sb[:, m : m + 1],
                            )
                            if ch == 0:
                                pps = pspool.tile([128, PQ], F32, name="pmm", tag="pmm", bufs=2)
                                for i in range(ND):
                                    nc.tensor.matmul(
                                        pps[:], wf16b[:, i, ts(mt, 128)], outT16p[:, i, :],
                                        start=(i == 0), stop=(i == ND - 1),
                                    )
                                nc.scalar.activation(
                                    st[:, 0:128], pps[:], AF.Gelu,
                                    bias=bf_sb[:, m : m + 1],
                                )
                            nc.sync.dma_start(ffT16[ts(m, 128), ts(ch, 512)], st[:])

    nc.compile()
    return nc


def _get_program():
    global _PROGRAM
    if _PROGRAM is None:
        _PROGRAM = _build_program()
    return _PROGRAM


def _owned_ranges(core):
    """(a0, b0): start rows of the two 512-token chunks core owns."""
    half = core % 2
    if half == 0:
        return 0, 3 * CH  # chunks 0, 3
    return CH, 2 * CH  # chunks 1, 2


def _pack(matT, nsl=ND):
    """[nsl*128, F] -> [128, nsl*F] so tile [128, nsl, F] loads in one DMA."""
    Dd, F = matT.shape
    assert Dd == nsl * 128
    return np.ascontiguousarray(
        matT.reshape(nsl, 128, F).transpose(1, 0, 2).reshape(128, nsl * F)
    )


def _q8(a, s):
    return (np.asarray(a, np.float32) * np.float32(s)).astype(F8NP)


def _make_in_maps(x, Wq, bq, Wk, bk, Wv, bv, Wo, bo, Wf, bf):
    f32 = np.float32
    f16 = np.float16
    bo2 = (Wo.astype(np.float64) @ bv.astype(np.float64) + bo.astype(np.float64)).astype(f32)

    def bias_col(v, scale=1.0):
        return np.ascontiguousarray((np.asarray(v, f32) * f32(scale)).reshape(-1, 128).T)

    wq8m = _q8(Wq.T, 32.0)
    wk8m = _q8(Wk.T, 32.0)
    wv8m = _q8(Wv.T, 32.0)
    wo8m = _q8(Wo.T, 32.0)
    wf8m = _q8(Wf.T, 32.0)  # [1024, 4096]
    iota = (
        np.arange(128, dtype=f32)[:, None]
        + 128.0 * np.arange(S // 128, dtype=f32)[None, :]
    )

    def pack_mb(mat):  # [1024, 4096] -> [128, 8*4096] mb-major
        return np.ascontiguousarray(
            mat.reshape(ND, 128, 8, 512).transpose(1, 2, 0, 3).reshape(128, ND * M)
        )

    shared = {
        "pwq8": _pack(wq8m), "pwk8": _pack(wk8m), "pwv8": _pack(wv8m),
        "pwo8": _pack(wo8m),
        "pwq16": _pack(Wq.T.astype(f16)), "pwk16": _pack(Wk.T.astype(f16)),
        "pwv16": _pack(Wv.T.astype(f16)), "pwo16": _pack(Wo.T.astype(f16)),
        "pwf8": pack_mb(wf8m), "pwf16": pack_mb(Wf.T.astype(f16)),
        "bq8": bias_col(bq, 32.0), "bk8": bias_col(bk, 32.0),
        "bq16": bias_col(bq), "bk16": bias_col(bk),
        "bo8": bias_col(bo2, 32.0), "bo16": bias_col(bo2),
        "bf32": bias_col(bf),
        "iota_kt": np.ascontiguousarray(iota),
    }
    in_maps = []
    for core in range(N_CORES):
        b = core // 2
        a0, b0 = _owned_ranges(core)
        xT = np.ascontiguousarray(x[b].T.astype(f32))  # [D, S]
        x8m = _q8(xT, 16.0)  # [1024, 2048]
        # tb-major pack: [128, (tb, i, t)]
        px8v = np.ascontiguousarray(
            x8m.reshape(ND, 128, 4, 512).transpose(1, 2, 0, 3).reshape(128, 4 * ND * 512)
        )
        xo8m = _q8(
            np.concatenate([xT[:, a0 : a0 + CH], xT[:, b0 : b0 + CH]], axis=1), 16.0
        )
        pxo8v = np.ascontiguousarray(
            xo8m.reshape(ND, 128, 2, 512).transpose(1, 2, 0, 3).reshape(128, 2 * ND * 512)
        )
        px16hv = _pack(xT[:, 0:PQ].astype(f16))
        pxo16hv = _pack(xT[:, a0 : a0 + PQ].astype(f16))
        qp = np.concatenate(
            [np.arange(a0, a0 + CH), np.arange(b0, b0 + CH)]
        ).astype(f32)[None, :]
        in_maps.append(
            {
                **shared,
                "px8": px8v, "pxo8": pxo8v,
                "px16h": px16hv, "pxo16h": pxo16hv,
                "qpos": np.ascontiguousarray(qp),
            }
        )
    return in_maps


def _run(inputs, trace=False, trace_cores=None, tmpdir=None):
    import sys

    if "/opt/trn_rl_repo" not in sys.path:
        sys.path.insert(0, "/opt/trn_rl_repo")
    from concourse.bass_utils import run_bass_kernel_spmd

    nc = _get_program()
    in_maps = _make_in_maps(**inputs)
    res = run_bass_kernel_spmd(
        nc, in_maps, list(range(N_CORES)), trace=trace, trace_cores=trace_cores,
        tmpdir=tmpdir,
    )
    out = np.empty((B, S, M), dtype=np.float32)
    for core in range(N_CORES):
        b = core // 2
        a0, b0 = _owned_ranges(core)
        ffT = np.asarray(res.results[core]["ffT16"], dtype=np.float32)  # [M, 1024]
        out[b, a0 : a0 + CH] = ffT[:, :CH].T
        out[b, b0 : b0 + CH] = ffT[:, CH:].T
    return out, res


def kernel(**inputs):
    out, _ = _run(inputs)
    return out
